# revision 40
# baseline (speedup 1.0000x reference)
"""MultiHeadedAttention block (B=4, S=2048, D=1024, H=16) on 8 TRN2 cores.

Sharding: core c handles batch b=c//2 and query-row half c%2 (1024 rows).
Each core computes full K/V projections for its batch, attention for all 16
heads over its 1024 query rows, then O-projection + residual + LayerNorm.
No collectives.

v3: fp8e4m3 + DoubleRow for all projections and P@V (scores stay bf16); the
Activation-engine exp stream is the bottleneck, so the schedule keeps it
saturated end to end:
  - Q-projection first, then K-projection for head pair 0, so the first
    exp fires ~12us in.
  - The V-projection is interleaved, two row-chunks per key-group, into
    head pair 0's qt0 attention: each Vt2[kg] tile is produced just before
    the P@V DoubleRow matmul that consumes it.
  - Head pairs 1..7 run a qt0 pass (with next-pair K-projections) then a
    qt1 pass with the first half of phase E (residual+LayerNorm rows that
    only need qt0 attention) interleaved; only the last 4 row-chunks of
    phase E trail the final exp.
Scale folding keeps fp8 exact-compensated:
  Wq x2048 (incl 1/sqrt(dk)), Wk x256 -- descale fused into the bias-add
  tensor_scalar; Wv x64 with ones-column 64 (cancels in the softmax
  normalization); Wo x256 with the residual pre-scaled x256 and LayerNorm
  run on the x256 domain with eps*256^2 (algebraically identical).
  bv enters via bo' = bo + Wo@bv (host, fp32, exact).
"""

import sys

if "/opt/trn_rl_repo" not in sys.path:
    sys.path.insert(0, "/opt/trn_rl_repo")

import ml_dtypes
import numpy as np

import concourse.bass as bass
import concourse.mybir as mybir
import concourse.tile as tile
from concourse.bass_utils import run_bass_kernel_spmd

B, S, D, H, DK = 4, 2048, 1024, 16, 64
P = 128
M = S // 2          # query rows per core
NDT = D // P        # 8 contraction chunks
NKP = NDT // 2      # 4 DoubleRow contraction pairs
NOT = D // P        # 8 output-feature chunks (= head pairs)
NHP = H // 2        # 8 head pairs
NKT = S // P        # 16 key chunks
NKG = NKT // 2      # 8 key-chunk pairs (DoubleRow P@V groups)
NQT = M // 512      # 2 query 512-chunks
NRT_K = S // 512    # 4 key-row 512-chunks
NRT_V = S // P      # 16 V row chunks
NRT_O = M // P      # 8 output row chunks
F32 = mybir.dt.float32
FP8 = mybir.dt.float8e4
BF16 = mybir.dt.bfloat16
MM_DT = mybir.dt.float32r
AF = mybir.ActivationFunctionType
ALU = mybir.AluOpType
DR = mybir.MatmulPerfMode.DoubleRow

SQ = 2048.0   # Wq (incl. 1/sqrt(dk)) storage scale
SK = 256.0    # Wk storage scale
SV = 64.0     # Wv storage scale (= ones-column value; cancels exactly)
SO = 256.0    # Wo storage scale (= residual pre-scale)


def _split_sync_waits(nc, max_waits=1):
    """Split instructions carrying more than max_waits sem waits.

    The container's walrus rejects instructions with multiple sync wait
    commands, so excess waits move onto NoOp instructions inserted just
    before, on the same engine.
    """
    idx = 0
    for f in nc.m.functions:
        for blk in f.blocks:
            newl = []
            for inst in blk.instructions:
                si = inst.sync_info
                waits = list(si.on_wait) if si is not None and si.on_wait else []
                if len(waits) > max_waits:
                    extra = waits[max_waits:]
                    si.on_wait = waits[:max_waits]
                    for j in range(0, len(extra), max_waits):
                        nop = mybir.InstNoOp(name=f"I-wsplit-{idx}", ins=[], outs=[])
                        idx += 1
                        nop.engine = inst.engine
                        nop.sync_info = mybir.SyncInfo(
                            on_wait=extra[j : j + max_waits], on_update=[]
                        )
                        newl.append(nop)
                newl.append(inst)
            blk.instructions = newl


def build_nc(loops=0):
    nc = bass.Bass()
    xqT = nc.dram_tensor("xqT", [D, M], FP8, kind="ExternalInput")
    xkT = nc.dram_tensor("xkT", [D, S], FP8, kind="ExternalInput")
    xvT = nc.dram_tensor("xvT", [D, S], FP8, kind="ExternalInput")
    qres = nc.dram_tensor("qres", [M, D], F32, kind="ExternalInput")
    WqT = nc.dram_tensor("WqT", [D, D], FP8, kind="ExternalInput")
    WkT = nc.dram_tensor("WkT", [D, D], FP8, kind="ExternalInput")
    WvT = nc.dram_tensor("WvT", [D, D], FP8, kind="ExternalInput")
    WoT = nc.dram_tensor("WoT", [D, D], FP8, kind="ExternalInput")
    bqv = nc.dram_tensor("bq", [D], F32, kind="ExternalInput")
    bkv = nc.dram_tensor("bk", [D], F32, kind="ExternalInput")
    gv = nc.dram_tensor("ln_g", [D], F32, kind="ExternalInput")
    bv2 = nc.dram_tensor("ln_b", [D], F32, kind="ExternalInput")
    onesf = nc.dram_tensor("onesf", [DK], F32, kind="ExternalInput")
    out = nc.dram_tensor("out", [M, D], BF16, kind="ExternalOutput")

    WqT_r = WqT[:, :].rearrange("(a p) o -> p a o", p=P)
    WkT_r = WkT[:, :].rearrange("(a p) o -> p a o", p=P)
    WvT_r = WvT[:, :].rearrange("(a p) o -> p a o", p=P)
    WoT_r = WoT[:, :].rearrange("(a p) o -> p a o", p=P)
    xqT_r = xqT[:, :].rearrange("(a p) r -> p a r", p=P)
    xkT_r = xkT[:, :].rearrange("(a p) r -> p a r", p=P)
    xvT_r = xvT[:, :].rearrange("(a p) r -> p a r", p=P)

    import contextlib

    with tile.TileContext(nc) as tc:
        loop_cm = tc.For_i(0, loops, 1) if loops else contextlib.nullcontext()
        loop_cm.__enter__()
        pxo_cm = tc.tile_pool(name="pxo", bufs=1)
        pxo = pxo_cm.__enter__()
        pqv_cm = tc.tile_pool(name="pqv", bufs=1)
        pqv = pqv_cm.__enter__()

        # ---- big input tiles, split so the first consumers start early.
        # scalar queue: xk halves (kproj0 is the very first PE work), then
        # xq halves.  sync queue: wk0, wq(ot0 slice), rest.
        xk_h = [
            pqv.tile([P, NDT, S // 2], FP8, tag=f"xk{h}", name=f"xk{h}")
            for h in range(2)
        ]
        xq_h = [
            pqv.tile([P, NDT, M // 2], FP8, tag=f"xq{h}", name=f"xq{h}")
            for h in range(2)
        ]
        nc.scalar.dma_start(xk_h[0], xkT_r[:, :, 0 : S // 2])
        nc.scalar.dma_start(xq_h[0], xqT_r[:, :, 0:512])
        nc.scalar.dma_start(xk_h[1], xkT_r[:, :, S // 2 : S])
        with tc.tile_wait_until(0.0075):
            nc.scalar.dma_start(xq_h[1], xqT_r[:, :, 512:M])
        pdw_cm = tc.tile_pool(name="pdw", bufs=2)
        pdw = pdw_cm.__enter__()
        wk01 = []
        wk0 = pdw.tile([P, NDT, P], FP8, tag="wk", name="wk")
        nc.sync.dma_start(wk0, WkT_r[:, :, 0:P])
        wk01.append(wk0)
        wq_a = pqv.tile([P, NDT, P], FP8, tag="wqa", name="wqa")
        nc.sync.dma_start(wq_a, WqT_r[:, :, 0:P])
        wk1 = pdw.tile([P, NDT, P], FP8, tag="wk", name="wk")
        nc.sync.dma_start(wk1, WkT_r[:, :, P : 2 * P])
        wk01.append(wk1)
        wq_b = pqv.tile([P, NDT, D - P], FP8, tag="wqb", name="wqb")
        with tc.tile_wait_until(0.009):
            nc.sync.dma_start(wq_b, WqT_r[:, :, P:D])
        xv_t = pqv.tile([P, NDT, S], FP8, tag="xv", name="xv")
        with tc.tile_wait_until(0.013):
            nc.sync.dma_start(xv_t, xvT_r)
        wv_t = pqv.tile([P, NDT, D], FP8, tag="wv", name="wv")
        with tc.tile_wait_until(0.019):
            nc.sync.dma_start(wv_t, WvT_r)
        # gpsimd queue: small/late things.
        bq_p = pqv.tile([P, NOT], F32)
        bk_p = pqv.tile([P, NOT], F32)
        cq_p = pqv.tile([P, 1], F32)
        ck_p = pqv.tile([P, 1], F32)
        nc.gpsimd.dma_start(bq_p, bqv[:].rearrange("(a p) -> p a", p=P))
        nc.gpsimd.dma_start(bk_p, bkv[:].rearrange("(a p) -> p a", p=P))
        nc.vector.memset(cq_p, 1.0 / SQ)
        nc.vector.memset(ck_p, 1.0 / SK)

        # attention output, feature-major: [pair features, pair, rows]
        XO = pxo.tile([P, NHP, M], FP8, tag="XO", name="XO")

        QT = {
            (ot, qt): pqv.tile(
                [P, 512], BF16, tag=f"QT{ot}_{qt}", name=f"QT{ot}_{qt}"
            )
            for ot in range(NOT)
            for qt in range(NQT)
        }
        # Paired V tiles for DoubleRow P@V: [128, kt-pair, head, DK+ones];
        # the ones-column (= SV) comes from cheap DVE memsets.
        Vt2 = []
        for kg in range(NKG):
            t = pqv.tile([P, 2, H, DK + 1], FP8, tag=f"Vt{kg}", name=f"Vt{kg}")
            nc.vector.memset(t[:, :, :, DK : DK + 1], SV)
            Vt2.append(t)

        pdkt_cm = tc.tile_pool(name="pdkt", bufs=NHP)
        pdkt = pdkt_cm.__enter__()

        kts = {}

        def kt_alloc(hp):
            kts[hp] = [
                pdkt.tile([P, 512], BF16, tag=f"kt{rt}", name="kt")
                for rt in range(NRT_K)
            ]
            return kts[hp]

        def kproj_part(hp, rt, pool, tag, wk):
            ps = pool.tile([P, 512], F32, tag=tag, name="kps")
            xkh = xk_h[rt // 2]
            col = (rt % 2) * 512
            for kp in range(NKP):
                nc.tensor.matmul(
                    ps,
                    wk[:, 2 * kp : 2 * kp + 2, :],
                    xkh[:, 2 * kp : 2 * kp + 2, col : col + 512],
                    start=(kp == 0),
                    stop=(kp == NKP - 1),
                    perf_mode=DR,
                )
            nc.vector.tensor_scalar(
                kts[hp][rt],
                ps,
                ck_p,
                bk_p[:, hp : hp + 1],
                op0=ALU.mult,
                op1=ALU.add,
            )

        def kproj(hp, pool, tag, wk=None):
            if wk is None:
                wk = pdw.tile([P, NDT, P], FP8, tag="wk", name="wk")
                nc.sync.dma_start(wk, WkT_r[:, :, hp * P : (hp + 1) * P])
            kt_alloc(hp)
            for rt in range(NRT_K):
                kproj_part(hp, rt, pool, tag, wk)

        # Scores PSUM banks are reserved first so they never alias the
        # projection pools (bank WAR edges would serialize the exp stream
        # behind deferred projection work).
        psS_cm = tc.tile_pool(name="psS", bufs=1, space="PSUM")
        psS = psS_cm.__enter__()

        # ---- Phase A (prefix): Q^T projection for head pair 0 only; the
        #      other 7 feature chunks ride inside hp0's qt0 exp slots.
        def apart(ot, qt, pool, tag):
            ps = pool.tile([P, 512], F32, tag=tag, name="ps")
            wq_s = (
                wq_a[:, :, :]
                if ot == 0
                else wq_b[:, :, (ot - 1) * P : ot * P]
            )
            for kp in range(NKP):
                nc.tensor.matmul(
                    ps,
                    wq_s[:, 2 * kp : 2 * kp + 2, :],
                    xq_h[qt][:, 2 * kp : 2 * kp + 2, :],
                    start=(kp == 0),
                    stop=(kp == NKP - 1),
                    perf_mode=DR,
                )
            nc.vector.tensor_scalar(
                QT[(ot, qt)],
                ps,
                cq_p,
                bq_p[:, ot : ot + 1],
                op0=ALU.mult,
                op1=ALU.add,
            )

        psA_cm = tc.tile_pool(name="psA", bufs=4, space="PSUM")
        psA = psA_cm.__enter__()
        kproj(0, psA, "ps", wk=wk01[0])
        apart(0, 0, psA, "ps")
        psA_cm.__exit__(None, None, None)

        # Phase-E constants (SBUF only; loaded during attention)
        pec_cm = tc.tile_pool(name="pec", bufs=1)
        pec = pec_cm.__enter__()
        g_b = pec.tile([P, D], BF16)
        b_b = pec.tile([P, D], BF16)
        eps_t = pec.tile([P, 1], F32)
        nc.vector.memset(eps_t, 1e-5 * SO * SO)

        # ---- Phase D scaffolding
        pdr_cm = tc.tile_pool(name="pdr", bufs=1)
        pdr = pdr_cm.__enter__()

        def bpart(rt):
            """V-projection for one 128-key row chunk (into Vt2[rt//2])."""
            ps = psB.tile([P, 512], F32, tag="bps", name="bps")
            for o2 in range(2):
                if o2 == 1:
                    ps = psB.tile([P, 512], F32, tag="bps", name="bps")
                for kp in range(NKP):
                    nc.tensor.matmul(
                        ps,
                        xv_t[:, 2 * kp : 2 * kp + 2, rt * P : (rt + 1) * P],
                        wv_t[:, 2 * kp : 2 * kp + 2, o2 * 512 : (o2 + 1) * 512],
                        start=(kp == 0),
                        stop=(kp == NKP - 1),
                        perf_mode=DR,
                    )
                nc.vector.tensor_copy(
                    Vt2[rt // 2][:, rt % 2, o2 * 8 : (o2 + 1) * 8, 0:DK],
                    ps[:, :].rearrange("p (h e) -> p h e", e=DK),
                )

        def scores_exp(hp, qt, kg, pool):
            """Scores + exp for one key-chunk pair; returns exs pair."""
            kt_h = kts[hp]
            sss = [
                psS.tile([P, 2, 512], F32, tag=f"ss{h01}", name=f"ss{h01}")
                for h01 in range(2)
            ]
            for j in range(2):
                kt = kg * 2 + j
                for h01 in range(2):
                    pb_ = h01 * DK
                    nc.tensor.matmul(
                        sss[h01][:, j, :],
                        kt_h[kt // 4][pb_ : pb_ + DK, (kt % 4) * P : (kt % 4 + 1) * P],
                        QT[(hp, qt)][pb_ : pb_ + DK, :],
                        start=True,
                        stop=True,
                        tile_position=(pb_, 0),
                    )
            exs = []
            for h01 in range(2):
                ex = pool.tile([P, 2, 512], FP8, tag=f"ex{h01}", name=f"ex{h01}")
                nc.scalar.activation(ex, sss[h01], AF.Exp)
                exs.append(ex)
            return exs

        def pv_mm(hp, kg, pv, exs):
            for h01 in range(2):
                nc.tensor.matmul(
                    pv[h01],
                    Vt2[kg][:, :, 2 * hp + h01, :],
                    exs[h01][:, :, :],
                    start=(kg == 0),
                    stop=(kg == NKG - 1),
                    perf_mode=DR,
                )

        def norm(hp, qt, pv):
            for h01 in range(2):
                pb_ = h01 * DK
                rc = pdr.tile([1, 512], MM_DT, tag="rc", name="rc")
                with nc.allow_low_precision(
                    reason="1/denom feeds f32r broadcast matmul"
                ):
                    nc.vector.reciprocal(rc, pv[h01][DK : DK + 1, :])
                rbp = psR.tile([DK, 512], F32, tag="rbp", name="rbp")
                nc.tensor.matmul(rbp, ones_t, rc, start=True, stop=True)
                rb_s = pdr.tile([DK, 512], F32, tag="rbs", name="rbs")
                nc.vector.tensor_copy(rb_s, rbp)
                dst = XO[pb_ : pb_ + DK, hp, qt * 512 : (qt + 1) * 512]
                nc.vector.tensor_mul(dst, pv[h01][0:DK, :], rb_s)

        def new_pv():
            return [
                psPV.tile([DK + 1, 512], F32, tag=f"pv{h01}", name=f"pv{h01}")
                for h01 in range(2)
            ]

        # -- head pair 0 (both qt): scores+exp only, exp outputs stashed;
        #    the V-projection streams through the same window.  P@V for
        #    pair 0 runs as a batch once V is complete.
        pde0_cm = tc.tile_pool(name="pde0", bufs=2 * NKG)
        pde0 = pde0_cm.__enter__()
        psB_cm = tc.tile_pool(name="psB", bufs=4, space="PSUM")
        psB = psB_cm.__enter__()
        # per-slot extra work inside head pair 0's exp stream:
        #   qt0 kg0..3: kproj(1) parts; kg2..7: Q-projection groups;
        #   kg4..7 + qt1 kg0..3: V-projection row pairs.
        a_sched = {(0, 0): [(0, 1)]}
        ots = [(ot, qt) for ot in range(1, NOT) for qt in range(NQT)]
        for i, (ot, qt) in enumerate(ots):
            a_sched.setdefault((0, 3 + i % 5), []).append((ot, qt))
        b_sched = {(0, 7): [0, 1]}
        for kg in range(7):
            b_sched[(1, kg)] = [2 + 2 * kg, 2 + 2 * kg + 1]
        psPV_cm = tc.tile_pool(name="psPV", bufs=1, space="PSUM")
        psR_cm = tc.tile_pool(name="psR", bufs=1, space="PSUM")
        psX_cm = tc.tile_pool(name="psX", bufs=1, space="PSUM")
        ones_t = pqv.tile([1, DK], MM_DT)
        wo_t = pqv.tile([P, NDT, D], FP8, tag="wo", name="wo")
        kt_alloc(1)
        exs0 = {}
        pv0q = {}
        for qt in range(NQT):
            for kg in range(NKG):
                slot = qt * NKG + kg
                if qt == 0 and kg < NRT_K:
                    with tc.tile_wait_until(0.012 + 0.002 * kg):
                        kproj_part(1, kg, psB, "bps", wk01[1])
                with tc.tile_wait_until(
                    0.013 if slot == 0 else 0.019 + 0.002 * slot
                ):
                    for ot, aqt in a_sched.get((qt, kg), []):
                        apart(ot, aqt, psB, "bps")
                with tc.tile_wait_until(0.027 + 0.0021 * (slot - 7)):
                    for rt in b_sched.get((qt, kg), []):
                        bpart(rt)
                exs0[(qt, kg)] = scores_exp(0, qt, kg, pde0)
                if qt == 1 and kg == 6:
                    nc.gpsimd.dma_start(
                        ones_t,
                        onesf[:].partition_broadcast(1).bitcast(MM_DT),
                    )
                if qt == 1 and kg == 7:
                    # V complete: swap psB banks for the P@V pools
                    psB_cm.__exit__(None, None, None)
                    psPV = psPV_cm.__enter__()
                    psR = psR_cm.__enter__()
                    psX = psX_cm.__enter__()
                    pv0q[0] = new_pv()
                    for k2 in range(NKG):
                        pv_mm(0, k2, pv0q[0], exs0[(0, k2)])

        norm(0, 0, pv0q[0])
        pv0q[1] = new_pv()
        for kg in range(NKG):
            pv_mm(0, kg, pv0q[1], exs0[(1, kg)])
        norm(0, 1, pv0q[1])
        pde0_cm.__exit__(None, None, None)

        pde_cm = tc.tile_pool(name="pde", bufs=4)
        pde = pde_cm.__enter__()

        # -- head pairs 1..7: qt0 pass (with next kprojs)
        for hp in range(1, NHP):
            if hp + 1 < NHP:
                kproj(hp + 1, psX, "kps")
            pv = new_pv()
            for kg in range(NKG):
                exs = scores_exp(hp, 0, kg, pde)
                pv_mm(hp, kg, pv, exs)
            norm(hp, 0, pv)

        # ---- Phase E helpers (on the x256 domain; see module docstring)
        peq_cm = tc.tile_pool(name="peq", bufs=4)
        peq = peq_cm.__enter__()
        pey_cm = tc.tile_pool(name="pey", bufs=6)
        pey = pey_cm.__enter__()
        pst_cm = tc.tile_pool(name="pst", bufs=8)
        pst = pst_cm.__enter__()

        eparts = {}

        def epartA(rt):
            """O-projection + residual + LN stats (no ACT instructions)."""
            qr = peq.tile([P, D], F32)
            nc.sync.dma_start(qr, qres[rt * P : (rt + 1) * P, :])
            y = pey.tile([P, D], BF16)
            for o2 in range(2):
                ps = psX.tile([P, 512], F32, tag="kps", name="eps")
                for kp in range(NKP):
                    nc.tensor.matmul(
                        ps,
                        XO[:, 2 * kp : 2 * kp + 2, rt * P : (rt + 1) * P],
                        wo_t[:, 2 * kp : 2 * kp + 2, o2 * 512 : (o2 + 1) * 512],
                        start=(kp == 0),
                        stop=(kp == NKP - 1),
                        perf_mode=DR,
                    )
                nc.vector.tensor_add(
                    y[:, o2 * 512 : (o2 + 1) * 512],
                    ps,
                    qr[:, o2 * 512 : (o2 + 1) * 512],
                )
            stats = pst.tile([P, 2, 6], F32)
            for sg in range(2):
                nc.vector.bn_stats(
                    stats[:, sg, :], y[:, sg * 512 : (sg + 1) * 512]
                )
            mv = pst.tile([P, 2], F32)
            nc.vector.bn_aggr(mv, stats)
            eparts[rt] = (y, mv)

        def epartB(rt):
            """Sqrt + normalize + gamma/beta + store (deferred past exps)."""
            y, mv = eparts.pop(rt)
            std = pst.tile([P, 1], F32)
            nc.scalar.activation(std, mv[:, 1:2], AF.Sqrt, bias=eps_t)
            rstd = pst.tile([P, 1], F32)
            nc.vector.reciprocal(rstd, std)
            nc.vector.tensor_scalar(
                y, y, mv[:, 0:1], rstd, op0=ALU.subtract, op1=ALU.mult
            )
            for o2 in range(2):
                sl = y[:, o2 * 512 : (o2 + 1) * 512]
                eng = nc.vector if o2 == 0 else nc.gpsimd
                eng.tensor_mul(sl, sl, g_b[:, o2 * 512 : (o2 + 1) * 512])
                eng.tensor_add(sl, sl, b_b[:, o2 * 512 : (o2 + 1) * 512])
            nc.sync.dma_start(out[rt * P : (rt + 1) * P, :], y)

        nc.gpsimd.dma_start(wo_t, WoT_r)
        nc.gpsimd.dma_start(g_b, gv[:].partition_broadcast(P))
        nc.gpsimd.dma_start(b_b, bv2[:].partition_broadcast(P))

        # -- head pairs 1..7: qt1 pass with early phase-E rows interleaved
        for hp in range(1, NHP):
            pv = new_pv()
            for kg in range(NKG):
                exs = scores_exp(hp, 1, kg, pde)
                pv_mm(hp, kg, pv, exs)
            norm(hp, 1, pv)
            if hp <= 4:
                epartA(hp - 1)

        for rt in range(4):
            epartB(rt)
        for rt in range(4, NRT_O):
            epartA(rt)
            if rt >= 5:
                epartB(rt - 1)
        epartB(NRT_O - 1)

        for cm in (
            pst_cm, pey_cm, peq_cm, pde_cm, psX_cm, psR_cm, psPV_cm,
            pdr_cm, psS_cm, pec_cm, pdkt_cm, pdw_cm, pqv_cm, pxo_cm,
        ):
            cm.__exit__(None, None, None)
        loop_cm.__exit__(None, None, None)
    _split_sync_waits(nc)
    return nc


_NC = None


def _get_nc():
    global _NC
    if _NC is None:
        _NC = build_nc()
    return _NC


def prepare_in_maps(q, k, v, Wq, bq, Wk, bk, Wv, bv, Wo, bo, ln_g, ln_b):
    f = np.float32
    f8 = ml_dtypes.float8_e4m3fn
    q = np.asarray(q, f)
    k = np.asarray(k, f)
    v = np.asarray(v, f)
    scale = 1.0 / np.sqrt(np.float32(DK))
    WqT = np.ascontiguousarray((np.asarray(Wq, f).T * (scale * SQ)).astype(f8))
    WkT = np.ascontiguousarray((np.asarray(Wk, f).T * SK).astype(f8))
    WvT = np.ascontiguousarray((np.asarray(Wv, f).T * SV).astype(f8))
    WoT = np.ascontiguousarray((np.asarray(Wo, f).T * SO).astype(f8))
    bq_s = np.asarray(bq, f) * scale
    bo_eff = np.asarray(bo, f) + np.asarray(Wo, f) @ np.asarray(bv, f)
    common = {
        "WqT": WqT,
        "WkT": WkT,
        "WvT": WvT,
        "WoT": WoT,
        "bq": bq_s,
        "bk": np.asarray(bk, f),
        "ln_g": np.asarray(ln_g, f),
        "ln_b": np.asarray(ln_b, f),
        "onesf": np.ones(DK, np.float32),
    }
    in_maps = []
    for c in range(8):
        b_, half = divmod(c, 2)
        qs = q[b_, half * M : (half + 1) * M, :]
        qres_c = (qs + bo_eff[None, :]) * SO
        in_maps.append(
            dict(
                common,
                xqT=np.ascontiguousarray(qs.T.astype(f8)),
                xkT=np.ascontiguousarray(k[b_].T.astype(f8)),
                xvT=np.ascontiguousarray(v[b_].T.astype(f8)),
                qres=np.ascontiguousarray(qres_c),
            )
        )
    return in_maps


def kernel(q, k, v, Wq, bq, Wk, bk, Wv, bv, Wo, bo, ln_g, ln_b):
    nc = _get_nc()
    in_maps = prepare_in_maps(q, k, v, Wq, bq, Wk, bk, Wv, bv, Wo, bo, ln_g, ln_b)
    res = run_bass_kernel_spmd(nc, in_maps, core_ids=list(range(8)))
    out = np.empty((B, S, D), np.float32)
    for c in range(8):
        b_, half = divmod(c, 2)
        out[b_, half * M : (half + 1) * M, :] = res.results[c]["out"]
    return out
